# revision 12
# baseline (speedup 1.0000x reference)
"""RQ-KMeans (residual vector quantization) Trainium2 kernel.

Problem: x [131072, 512] f32, codebooks [4, 1024, 512] f32.
Per book b: idx_b = argmin_k ||res - C_b[k]||^2 ; res -= C_b[idx_b].
Returns (idx [131072, 4] int32, res [131072, 512] f32).

Strategy: data-parallel over rows across 8 NeuronCores (16384 rows each),
codebooks replicated. Book-major on-device loop: for each book, stream row
tiles of the residual from DRAM, compute scores via PE matmul
(res . C^T - |C|^2/2), per-row argmax on DVE (max/max_index), gather the
selected code rows with indirect DMA, subtract, store the updated residual.
argmin(d2) == argmax(res.C - |C|^2/2) since the |res|^2 term is constant
per row.
"""

import sys

for _p in ("/root/.axon_site", "/root/.axon_site/_ro/trn_rl_repo", "/opt/trn_rl_repo"):
    if _p not in sys.path:
        sys.path.append(_p)

import numpy as np

import concourse.bass as bass
import concourse.mybir as mybir
import concourse.tile as tile
from concourse import bacc, bass_utils
from concourse.bass import ds
from concourse.bass_interp import get_hw_module
from concourse.masks import make_identity

N_CORES = 8
D = 512
K = 1024
NUM_BOOK = 4
P = 128
DCH = D // P  # 4 contraction chunks
KH = K // 512  # rhs free-dim halves (fp32 moving operand max is 512)

UNROLL = 4  # row tiles per For_i iteration
MM_DTYPE = mybir.dt.float32

_cache = {}


def _build(nc_rows: int):
    """Build + compile the per-core Bass program for nc_rows rows."""
    ntiles = nc_rows // P
    unroll = min(UNROLL, ntiles)
    assert ntiles % unroll == 0

    import time as _time

    _t0 = _time.time()
    nc = bacc.Bacc("TRN2", target_bir_lowering=False, debug=False)
    f32 = mybir.dt.float32

    x_d = nc.dram_tensor("x", [nc_rows, D], f32, kind="ExternalInput").ap()
    cbt_d = nc.dram_tensor("cbt", [NUM_BOOK, D, K], f32, kind="ExternalInput").ap()
    # [P, NUM_BOOK, K]: matches the SBUF tile layout (row-broadcast bias)
    cnorm_d = nc.dram_tensor("cnorm", [P, NUM_BOOK, K], f32, kind="ExternalInput").ap()
    # separate codebook tensors for the indirect gather (its source AP must
    # have offset 0, so one tensor per book)
    cbn_d = [
        nc.dram_tensor(f"cb{b}", [K, D], f32, kind="ExternalInput").ap()
        for b in range(NUM_BOOK)
    ]
    out_res_d = nc.dram_tensor("out_res", [nc_rows, D], f32, kind="ExternalOutput").ap()
    out_idx_d = nc.dram_tensor(
        "out_idx", [nc_rows, NUM_BOOK], mybir.dt.int32, kind="ExternalOutput"
    ).ap()
    res_scratch = [
        nc.dram_tensor(f"res_scratch{j}", [nc_rows, D], f32, kind="Internal").ap()
        for j in range(2)
    ]

    with tile.TileContext(nc, trace_sim=False) as tc:
        with (
            tc.tile_pool(name="const", bufs=1) as const_pool,
            tc.tile_pool(name="cbt", bufs=2) as cbt_pool,
            tc.tile_pool(name="work", bufs=2 * unroll) as work,
            tc.tile_pool(name="psum_t", bufs=2, space="PSUM") as psum_t,
            tc.tile_pool(name="psum_p", bufs=2, space="PSUM") as psum_p,
        ):
            ident = const_pool.tile([P, P], f32)
            make_identity(nc, ident[:])
            cnorm_sb = const_pool.tile([P, NUM_BOOK, K], f32)
            nc.sync.dma_start(cnorm_sb[:], cnorm_d)

            def body(b, src, dst, cbt_sb, row0):
                res_in = work.tile([P, D], f32, tag="res_in")
                nc.sync.dma_start(res_in[:], src[ds(row0, P), :])

                # resT in PSUM via 4 PE transposes
                resT_ps = psum_t.tile([P, D], f32, tag="resT")
                for c in range(DCH):
                    nc.tensor.transpose(
                        resT_ps[:, c * P : (c + 1) * P],
                        res_in[:, c * P : (c + 1) * P],
                        ident[:],
                    )
                resT_sb = work.tile([P, D], MM_DTYPE, tag="resT_sb")
                nc.scalar.copy(resT_sb[:], resT_ps[:])

                # scores: P_ps[n, k] = res_n . C_k  (accumulate over 4 chunks)
                p_ps = psum_p.tile([P, K], f32, tag="p")
                for c in range(DCH):
                    for h in range(KH):
                        nc.tensor.matmul(
                            out=p_ps[:, h * 512 : (h + 1) * 512],
                            lhsT=resT_sb[:, c * P : (c + 1) * P],
                            rhs=cbt_sb[:, c, h * 512 : (h + 1) * 512],
                            start=(c == 0),
                            stop=(c == DCH - 1),
                        )

                # m = P - |C|^2/2 ; argmax over k
                m_sb = work.tile([P, K], f32, tag="m")
                nc.vector.tensor_sub(m_sb[:], p_ps[:], cnorm_sb[:, b, :])
                mx8 = work.tile([P, 8], f32, tag="mx8")
                nc.vector.max(out=mx8[:], in_=m_sb[:])
                idx8 = work.tile([P, 8], mybir.dt.uint32, tag="idx8")
                nc.vector.max_index(idx8[:], mx8[:], m_sb[:])

                # gather selected code rows and subtract
                csel = work.tile([P, D], f32, tag="csel")
                nc.gpsimd.indirect_dma_start(
                    out=csel[:],
                    out_offset=None,
                    in_=cbn_d[b],
                    in_offset=bass.IndirectOffsetOnAxis(ap=idx8[:, :1], axis=0),
                )
                res_out = work.tile([P, D], f32, tag="res_out")
                nc.vector.tensor_sub(res_out[:], res_in[:], csel[:])

                nc.sync.dma_start(dst[ds(row0, P), :], res_out[:])
                nc.sync.dma_start(
                    out_idx_d[ds(row0, P), b : b + 1],
                    idx8[:, :1].bitcast(mybir.dt.int32),
                )

            for b in range(NUM_BOOK):
                src = x_d if b == 0 else res_scratch[(b - 1) % 2]
                dst = out_res_d if b == NUM_BOOK - 1 else res_scratch[b % 2]
                cbt_sb = cbt_pool.tile([P, DCH, K], MM_DTYPE, tag="cbt_sb")
                nc.sync.dma_start(
                    cbt_sb[:], cbt_d[b].rearrange("(c p) k -> p c k", p=P)
                )
                if ntiles == unroll:
                    for u in range(unroll):
                        body(b, src, dst, cbt_sb, u * P)
                else:
                    with tc.For_i(0, ntiles * P, unroll * P) as row0:
                        for u in range(unroll):
                            body(b, src, dst, cbt_sb, row0 + u * P)

    import time as _time

    _t = _time.time()
    nc.compile()
    print(f"[build] bacc compile: {_time.time()-_t:.1f}s", flush=True)
    nc.m = get_hw_module(nc.m)
    return nc


def _cnorm_like_reference(codebooks: np.ndarray) -> np.ndarray:
    """|C_k|^2 computed to match the reference's jnp.sum(C*C, -1) on XLA-CPU
    bit-for-bit (the reference can only run on CPU: its argmin reduce doesn't
    compile for the neuron device). Runs in a clean subprocess so it works
    even when the calling process's jax is locked to the axon platform."""
    import subprocess, tempfile, os

    try:
        with tempfile.TemporaryDirectory() as td:
            src = os.path.join(td, "cb.npy")
            dst = os.path.join(td, "cn.npy")
            np.save(src, codebooks)
            env = {k: v for k, v in os.environ.items() if k != "TRN_TERMINAL_POOL_IPS"}
            env["JAX_PLATFORMS"] = "cpu"
            code = (
                "import numpy as np, jax.numpy as jnp;"
                f"c = jnp.asarray(np.load({src!r}));"
                f"np.save({dst!r}, np.asarray(jnp.sum(c * c, axis=-1), dtype=np.float32))"
            )
            subprocess.run(
                [sys.executable, "-c", code], env=env, check=True,
                capture_output=True, timeout=300,
            )
            return np.load(dst)
    except Exception:
        pass
    try:
        import jax
        import jax.numpy as jnp

        cpu = jax.devices("cpu")[0]
        with jax.default_device(cpu):
            cb_j = jax.device_put(codebooks, cpu)
            return np.asarray(jnp.sum(cb_j * cb_j, axis=-1), dtype=np.float32)
    except Exception:
        return np.sum(codebooks.astype(np.float32) ** 2, axis=-1, dtype=np.float32)


def _prep_inputs(x: np.ndarray, codebooks: np.ndarray):
    n = x.shape[0]
    nc_rows = n // N_CORES
    x = np.ascontiguousarray(x, dtype=np.float32)
    codebooks = np.ascontiguousarray(codebooks, dtype=np.float32)
    cbt = np.ascontiguousarray(codebooks.transpose(0, 2, 1))  # [B, D, K]
    cn = _cnorm_like_reference(codebooks)
    cnorm = np.ascontiguousarray(
        np.broadcast_to((0.5 * cn)[None, :, :], (P, NUM_BOOK, K)), dtype=np.float32
    )  # [P, B, K] — matches the SBUF tile layout
    in_maps = []
    for c in range(N_CORES):
        m = {
            "x": x[c * nc_rows : (c + 1) * nc_rows],
            "cbt": cbt,
            "cnorm": cnorm,
        }
        for b in range(NUM_BOOK):
            m[f"cb{b}"] = codebooks[b]
        in_maps.append(m)
    return in_maps, nc_rows


def kernel(x: np.ndarray, codebooks: np.ndarray, _trace=False):
    in_maps, nc_rows = _prep_inputs(x, codebooks)
    if nc_rows not in _cache:
        _cache[nc_rows] = _build(nc_rows)
    nc = _cache[nc_rows]
    res = bass_utils.run_bass_kernel_spmd(
        nc, in_maps, core_ids=list(range(N_CORES)), trace=_trace
    )
    idx = np.concatenate([r["out_idx"] for r in res.results], axis=0)
    resid = np.concatenate([r["out_res"] for r in res.results], axis=0)
    out = (idx.astype(np.int32), resid)
    if _trace:
        return out, res
    return out


# revision 13
# speedup vs baseline: 1.3759x; 1.3759x over previous
"""RQ-KMeans (residual vector quantization) Trainium2 kernel.

Problem: x [131072, 512] f32, codebooks [4, 1024, 512] f32.
Per book b: idx_b = argmin_k ||res - C_b[k]||^2 ; res -= C_b[idx_b].
Returns (idx [131072, 4] int32, res [131072, 512] f32).

Strategy: data-parallel over rows across 8 NeuronCores (16384 rows each),
codebooks replicated. Book-major on-device loop: for each book, stream row
tiles of the residual from DRAM, compute scores via PE matmul
(res . C^T - |C|^2/2), per-row argmax on DVE (max/max_index), gather the
selected code rows with indirect DMA, subtract, store the updated residual.
argmin(d2) == argmax(res.C - |C|^2/2) since the |res|^2 term is constant
per row.
"""

import sys

for _p in ("/root/.axon_site", "/root/.axon_site/_ro/trn_rl_repo", "/opt/trn_rl_repo"):
    if _p not in sys.path:
        sys.path.append(_p)

import numpy as np

import concourse.bass as bass
import concourse.mybir as mybir
import concourse.tile as tile
from concourse import bacc, bass_utils
from concourse.bass import ds
from concourse.bass_interp import get_hw_module
from concourse.masks import make_identity

N_CORES = 8
D = 512
K = 1024
NUM_BOOK = 4
P = 128
DCH = D // P  # 4 contraction chunks
KH = K // 512  # rhs free-dim halves (fp32 moving operand max is 512)

UNROLL = 16  # row tiles per For_i iteration
MM_DTYPE = mybir.dt.float32

_cache = {}


def _build(nc_rows: int):
    """Build + compile the per-core Bass program for nc_rows rows."""
    ntiles = nc_rows // P
    unroll = min(UNROLL, ntiles)
    assert ntiles % unroll == 0

    import time as _time

    _t0 = _time.time()
    nc = bacc.Bacc("TRN2", target_bir_lowering=False, debug=False)
    f32 = mybir.dt.float32

    x_d = nc.dram_tensor("x", [nc_rows, D], f32, kind="ExternalInput").ap()
    cbt_d = nc.dram_tensor("cbt", [NUM_BOOK, D, K], f32, kind="ExternalInput").ap()
    # [P, NUM_BOOK, K]: matches the SBUF tile layout (row-broadcast bias)
    cnorm_d = nc.dram_tensor("cnorm", [P, NUM_BOOK, K], f32, kind="ExternalInput").ap()
    # separate codebook tensors for the indirect gather (its source AP must
    # have offset 0, so one tensor per book)
    cbn_d = [
        nc.dram_tensor(f"cb{b}", [K, D], f32, kind="ExternalInput").ap()
        for b in range(NUM_BOOK)
    ]
    out_res_d = nc.dram_tensor("out_res", [nc_rows, D], f32, kind="ExternalOutput").ap()
    out_idx_d = nc.dram_tensor(
        "out_idx", [nc_rows, NUM_BOOK], mybir.dt.int32, kind="ExternalOutput"
    ).ap()
    res_scratch = [
        nc.dram_tensor(f"res_scratch{j}", [nc_rows, D], f32, kind="Internal").ap()
        for j in range(2)
    ]

    with tile.TileContext(nc, trace_sim=False) as tc:
        with (
            tc.tile_pool(name="const", bufs=1) as const_pool,
            tc.tile_pool(name="cbt", bufs=2) as cbt_pool,
            tc.tile_pool(name="work", bufs=min(10, 2 * unroll)) as work,
            tc.tile_pool(name="psum_t", bufs=2, space="PSUM") as psum_t,
            tc.tile_pool(name="psum_p", bufs=3, space="PSUM") as psum_p,
        ):
            ident = const_pool.tile([P, P], f32)
            make_identity(nc, ident[:])
            cnorm_sb = const_pool.tile([P, NUM_BOOK, K], f32)
            nc.sync.dma_start(cnorm_sb[:], cnorm_d)

            def body(b, src, dst, cbt_sb, row0):
                res_in = work.tile([P, D], f32, tag="res_in")
                nc.sync.dma_start(res_in[:], src[ds(row0, P), :])

                # resT in PSUM via 4 PE transposes
                resT_ps = psum_t.tile([P, D], f32, tag="resT")
                for c in range(DCH):
                    nc.tensor.transpose(
                        resT_ps[:, c * P : (c + 1) * P],
                        res_in[:, c * P : (c + 1) * P],
                        ident[:],
                    )
                resT_sb = work.tile([P, D], MM_DTYPE, tag="resT_sb")
                nc.scalar.copy(resT_sb[:], resT_ps[:])

                # scores: P_ps[n, k] = res_n . C_k  (accumulate over 4 chunks)
                p_ps = psum_p.tile([P, K], f32, tag="p")
                for c in range(DCH):
                    for h in range(KH):
                        nc.tensor.matmul(
                            out=p_ps[:, h * 512 : (h + 1) * 512],
                            lhsT=resT_sb[:, c * P : (c + 1) * P],
                            rhs=cbt_sb[:, c, h * 512 : (h + 1) * 512],
                            start=(c == 0),
                            stop=(c == DCH - 1),
                        )

                # m = P - |C|^2/2 ; argmax over k
                m_sb = work.tile([P, K], f32, tag="m")
                nc.vector.tensor_sub(m_sb[:], p_ps[:], cnorm_sb[:, b, :])
                mx8 = work.tile([P, 8], f32, tag="mx8")
                nc.vector.max(out=mx8[:], in_=m_sb[:])
                idx8 = work.tile([P, 8], mybir.dt.uint32, tag="idx8")
                nc.vector.max_index(idx8[:], mx8[:], m_sb[:])

                # gather selected code rows and subtract
                csel = work.tile([P, D], f32, tag="csel")
                nc.gpsimd.indirect_dma_start(
                    out=csel[:],
                    out_offset=None,
                    in_=cbn_d[b],
                    in_offset=bass.IndirectOffsetOnAxis(ap=idx8[:, :1], axis=0),
                )
                res_out = work.tile([P, D], f32, tag="res_out")
                nc.vector.tensor_sub(res_out[:], res_in[:], csel[:])

                nc.sync.dma_start(dst[ds(row0, P), :], res_out[:])
                nc.sync.dma_start(
                    out_idx_d[ds(row0, P), b : b + 1],
                    idx8[:, :1].bitcast(mybir.dt.int32),
                )

            for b in range(NUM_BOOK):
                src = x_d if b == 0 else res_scratch[(b - 1) % 2]
                dst = out_res_d if b == NUM_BOOK - 1 else res_scratch[b % 2]
                cbt_sb = cbt_pool.tile([P, DCH, K], MM_DTYPE, tag="cbt_sb")
                nc.sync.dma_start(
                    cbt_sb[:], cbt_d[b].rearrange("(c p) k -> p c k", p=P)
                )
                if ntiles == unroll:
                    for u in range(unroll):
                        body(b, src, dst, cbt_sb, u * P)
                else:
                    with tc.For_i(0, ntiles * P, unroll * P) as row0:
                        for u in range(unroll):
                            body(b, src, dst, cbt_sb, row0 + u * P)

    import time as _time

    _t = _time.time()
    nc.compile()
    print(f"[build] bacc compile: {_time.time()-_t:.1f}s", flush=True)
    nc.m = get_hw_module(nc.m)
    return nc


def _cnorm_like_reference(codebooks: np.ndarray) -> np.ndarray:
    """|C_k|^2 computed to match the reference's jnp.sum(C*C, -1) on XLA-CPU
    bit-for-bit (the reference can only run on CPU: its argmin reduce doesn't
    compile for the neuron device). Runs in a clean subprocess so it works
    even when the calling process's jax is locked to the axon platform."""
    import subprocess, tempfile, os

    try:
        with tempfile.TemporaryDirectory() as td:
            src = os.path.join(td, "cb.npy")
            dst = os.path.join(td, "cn.npy")
            np.save(src, codebooks)
            env = {k: v for k, v in os.environ.items() if k != "TRN_TERMINAL_POOL_IPS"}
            env["JAX_PLATFORMS"] = "cpu"
            code = (
                "import numpy as np, jax.numpy as jnp;"
                f"c = jnp.asarray(np.load({src!r}));"
                f"np.save({dst!r}, np.asarray(jnp.sum(c * c, axis=-1), dtype=np.float32))"
            )
            subprocess.run(
                [sys.executable, "-c", code], env=env, check=True,
                capture_output=True, timeout=300,
            )
            return np.load(dst)
    except Exception:
        pass
    try:
        import jax
        import jax.numpy as jnp

        cpu = jax.devices("cpu")[0]
        with jax.default_device(cpu):
            cb_j = jax.device_put(codebooks, cpu)
            return np.asarray(jnp.sum(cb_j * cb_j, axis=-1), dtype=np.float32)
    except Exception:
        return np.sum(codebooks.astype(np.float32) ** 2, axis=-1, dtype=np.float32)


def _prep_inputs(x: np.ndarray, codebooks: np.ndarray):
    n = x.shape[0]
    nc_rows = n // N_CORES
    x = np.ascontiguousarray(x, dtype=np.float32)
    codebooks = np.ascontiguousarray(codebooks, dtype=np.float32)
    cbt = np.ascontiguousarray(codebooks.transpose(0, 2, 1))  # [B, D, K]
    cn = _cnorm_like_reference(codebooks)
    cnorm = np.ascontiguousarray(
        np.broadcast_to((0.5 * cn)[None, :, :], (P, NUM_BOOK, K)), dtype=np.float32
    )  # [P, B, K] — matches the SBUF tile layout
    in_maps = []
    for c in range(N_CORES):
        m = {
            "x": x[c * nc_rows : (c + 1) * nc_rows],
            "cbt": cbt,
            "cnorm": cnorm,
        }
        for b in range(NUM_BOOK):
            m[f"cb{b}"] = codebooks[b]
        in_maps.append(m)
    return in_maps, nc_rows


def kernel(x: np.ndarray, codebooks: np.ndarray, _trace=False):
    in_maps, nc_rows = _prep_inputs(x, codebooks)
    if nc_rows not in _cache:
        _cache[nc_rows] = _build(nc_rows)
    nc = _cache[nc_rows]
    res = bass_utils.run_bass_kernel_spmd(
        nc, in_maps, core_ids=list(range(N_CORES)), trace=_trace
    )
    idx = np.concatenate([r["out_idx"] for r in res.results], axis=0)
    resid = np.concatenate([r["out_res"] for r in res.results], axis=0)
    out = (idx.astype(np.int32), resid)
    if _trace:
        return out, res
    return out


# revision 15
# speedup vs baseline: 1.4732x; 1.0708x over previous
"""RQ-KMeans (residual vector quantization) Trainium2 kernel.

Problem: x [131072, 512] f32, codebooks [4, 1024, 512] f32.
Per book b: idx_b = argmin_k ||res - C_b[k]||^2 ; res -= C_b[idx_b].
Returns (idx [131072, 4] int32, res [131072, 512] f32).

Strategy: data-parallel over rows across 8 NeuronCores (16384 rows each),
codebooks replicated. Book-major on-device loop: for each book, stream row
tiles of the residual from DRAM, compute scores via PE matmul
(res . C^T - |C|^2/2), per-row argmax on DVE (max/max_index), gather the
selected code rows with indirect DMA, subtract, store the updated residual.
argmin(d2) == argmax(res.C - |C|^2/2) since the |res|^2 term is constant
per row.
"""

import sys

for _p in ("/root/.axon_site", "/root/.axon_site/_ro/trn_rl_repo", "/opt/trn_rl_repo"):
    if _p not in sys.path:
        sys.path.append(_p)

import numpy as np

import concourse.bass as bass
import concourse.mybir as mybir
import concourse.tile as tile
from concourse import bacc, bass_utils
from concourse.bass import ds
from concourse.bass_interp import get_hw_module
from concourse.masks import make_identity

N_CORES = 8
D = 512
K = 1024
NUM_BOOK = 4
P = 128
DCH = D // P  # 4 contraction chunks
KH = K // 512  # rhs free-dim halves (fp32 moving operand max is 512)

UNROLL = 32  # row tiles per For_i iteration
MM_DTYPE = mybir.dt.float32
# 3-pass float32r split matmul: res.C ~= rh.Ch + rh.Cl + rl.Ch with rh/rl
# (Ch/Cl) the f32r-rounded value and remainder. f32r streams 1 col/cycle vs
# fp32's 4, so 3 passes beat 1 fp32 pass by ~25%; dropped rl.Cl term is
# ~2^-22 relative — below fp32 accumulation-order noise.
SPLIT3 = bool(int(__import__("os").environ.get("RQ_SPLIT3", "0")))

_cache = {}


def _build(nc_rows: int):
    """Build + compile the per-core Bass program for nc_rows rows."""
    ntiles = nc_rows // P
    unroll = min(UNROLL, ntiles)
    assert ntiles % unroll == 0

    import time as _time

    _t0 = _time.time()
    nc = bacc.Bacc("TRN2", target_bir_lowering=False, debug=False)
    f32 = mybir.dt.float32

    x_d = nc.dram_tensor("x", [nc_rows, D], f32, kind="ExternalInput").ap()
    cbt_d = nc.dram_tensor("cbt", [NUM_BOOK, D, K], f32, kind="ExternalInput").ap()
    # [P, NUM_BOOK, K]: matches the SBUF tile layout (row-broadcast bias)
    cnorm_d = nc.dram_tensor("cnorm", [P, NUM_BOOK, K], f32, kind="ExternalInput").ap()
    # separate codebook tensors for the indirect gather (its source AP must
    # have offset 0, so one tensor per book)
    cbn_d = [
        nc.dram_tensor(f"cb{b}", [K, D], f32, kind="ExternalInput").ap()
        for b in range(NUM_BOOK)
    ]
    out_res_d = nc.dram_tensor("out_res", [nc_rows, D], f32, kind="ExternalOutput").ap()
    out_idx_d = nc.dram_tensor(
        "out_idx", [nc_rows, NUM_BOOK], mybir.dt.int32, kind="ExternalOutput"
    ).ap()
    res_scratch = [
        nc.dram_tensor(f"res_scratch{j}", [nc_rows, D], f32, kind="Internal").ap()
        for j in range(2)
    ]

    with tile.TileContext(nc, trace_sim=False) as tc:
        with (
            tc.tile_pool(name="const", bufs=1) as const_pool,
            tc.tile_pool(name="cbt", bufs=2) as cbt_pool,
            tc.tile_pool(name="work", bufs=min(10, 2 * unroll)) as work,
            tc.tile_pool(name="psum_t", bufs=2, space="PSUM") as psum_t,
            tc.tile_pool(name="psum_p", bufs=3, space="PSUM") as psum_p,
        ):
            ident = const_pool.tile([P, P], f32)
            make_identity(nc, ident[:])
            cnorm_sb = const_pool.tile([P, NUM_BOOK, K], f32)
            nc.sync.dma_start(cnorm_sb[:], cnorm_d)

            def body(b, src, dst, cbt_sb, row0):
                res_in = work.tile([P, D], f32, tag="res_in")
                nc.sync.dma_start(res_in[:], src[ds(row0, P), :])

                # resT in PSUM via 4 PE transposes
                resT_ps = psum_t.tile([P, D], f32, tag="resT")
                for c in range(DCH):
                    nc.tensor.transpose(
                        resT_ps[:, c * P : (c + 1) * P],
                        res_in[:, c * P : (c + 1) * P],
                        ident[:],
                    )
                resT_sb = work.tile([P, D], MM_DTYPE, tag="resT_sb")
                nc.scalar.copy(resT_sb[:], resT_ps[:])

                # scores: P_ps[n, k] = res_n . C_k  (accumulate over 4 chunks)
                p_ps = psum_p.tile([P, K], f32, tag="p")
                for c in range(DCH):
                    for h in range(KH):
                        nc.tensor.matmul(
                            out=p_ps[:, h * 512 : (h + 1) * 512],
                            lhsT=resT_sb[:, c * P : (c + 1) * P],
                            rhs=cbt_sb[:, c, h * 512 : (h + 1) * 512],
                            start=(c == 0),
                            stop=(c == DCH - 1),
                        )

                # m = P - |C|^2/2 ; argmax over k
                m_sb = work.tile([P, K], f32, tag="m")
                nc.vector.tensor_sub(m_sb[:], p_ps[:], cnorm_sb[:, b, :])
                mx8 = work.tile([P, 8], f32, tag="mx8")
                nc.vector.max(out=mx8[:], in_=m_sb[:])
                idx8 = work.tile([P, 8], mybir.dt.uint32, tag="idx8")
                nc.vector.max_index(idx8[:], mx8[:], m_sb[:])

                # gather selected code rows and subtract
                csel = work.tile([P, D], f32, tag="csel")
                nc.gpsimd.indirect_dma_start(
                    out=csel[:],
                    out_offset=None,
                    in_=cbn_d[b],
                    in_offset=bass.IndirectOffsetOnAxis(ap=idx8[:, :1], axis=0),
                )
                res_out = work.tile([P, D], f32, tag="res_out")
                nc.vector.tensor_sub(res_out[:], res_in[:], csel[:])

                nc.sync.dma_start(dst[ds(row0, P), :], res_out[:])
                nc.sync.dma_start(
                    out_idx_d[ds(row0, P), b : b + 1],
                    idx8[:, :1].bitcast(mybir.dt.int32),
                )

            for b in range(NUM_BOOK):
                src = x_d if b == 0 else res_scratch[(b - 1) % 2]
                dst = out_res_d if b == NUM_BOOK - 1 else res_scratch[b % 2]
                cbt_sb = cbt_pool.tile([P, DCH, K], MM_DTYPE, tag="cbt_sb")
                nc.sync.dma_start(
                    cbt_sb[:], cbt_d[b].rearrange("(c p) k -> p c k", p=P)
                )
                if ntiles == unroll:
                    for u in range(unroll):
                        body(b, src, dst, cbt_sb, u * P)
                else:
                    with tc.For_i(0, ntiles * P, unroll * P) as row0:
                        for u in range(unroll):
                            body(b, src, dst, cbt_sb, row0 + u * P)

    import time as _time

    _t = _time.time()
    nc.compile()
    print(f"[build] bacc compile: {_time.time()-_t:.1f}s", flush=True)
    nc.m = get_hw_module(nc.m)
    return nc


def _cnorm_like_reference(codebooks: np.ndarray) -> np.ndarray:
    """|C_k|^2 computed to match the reference's jnp.sum(C*C, -1) on XLA-CPU
    bit-for-bit (the reference can only run on CPU: its argmin reduce doesn't
    compile for the neuron device). Runs in a clean subprocess so it works
    even when the calling process's jax is locked to the axon platform."""
    import subprocess, tempfile, os

    try:
        with tempfile.TemporaryDirectory() as td:
            src = os.path.join(td, "cb.npy")
            dst = os.path.join(td, "cn.npy")
            np.save(src, codebooks)
            env = {k: v for k, v in os.environ.items() if k != "TRN_TERMINAL_POOL_IPS"}
            env["JAX_PLATFORMS"] = "cpu"
            code = (
                "import numpy as np, jax.numpy as jnp;"
                f"c = jnp.asarray(np.load({src!r}));"
                f"np.save({dst!r}, np.asarray(jnp.sum(c * c, axis=-1), dtype=np.float32))"
            )
            subprocess.run(
                [sys.executable, "-c", code], env=env, check=True,
                capture_output=True, timeout=300,
            )
            return np.load(dst)
    except Exception:
        pass
    try:
        import jax
        import jax.numpy as jnp

        cpu = jax.devices("cpu")[0]
        with jax.default_device(cpu):
            cb_j = jax.device_put(codebooks, cpu)
            return np.asarray(jnp.sum(cb_j * cb_j, axis=-1), dtype=np.float32)
    except Exception:
        return np.sum(codebooks.astype(np.float32) ** 2, axis=-1, dtype=np.float32)


def _prep_inputs(x: np.ndarray, codebooks: np.ndarray):
    n = x.shape[0]
    nc_rows = n // N_CORES
    x = np.ascontiguousarray(x, dtype=np.float32)
    codebooks = np.ascontiguousarray(codebooks, dtype=np.float32)
    cbt = np.ascontiguousarray(codebooks.transpose(0, 2, 1))  # [B, D, K]
    cn = _cnorm_like_reference(codebooks)
    cnorm = np.ascontiguousarray(
        np.broadcast_to((0.5 * cn)[None, :, :], (P, NUM_BOOK, K)), dtype=np.float32
    )  # [P, B, K] — matches the SBUF tile layout
    in_maps = []
    for c in range(N_CORES):
        m = {
            "x": x[c * nc_rows : (c + 1) * nc_rows],
            "cbt": cbt,
            "cnorm": cnorm,
        }
        for b in range(NUM_BOOK):
            m[f"cb{b}"] = codebooks[b]
        in_maps.append(m)
    return in_maps, nc_rows


def kernel(x: np.ndarray, codebooks: np.ndarray, _trace=False):
    in_maps, nc_rows = _prep_inputs(x, codebooks)
    if nc_rows not in _cache:
        _cache[nc_rows] = _build(nc_rows)
    nc = _cache[nc_rows]
    res = bass_utils.run_bass_kernel_spmd(
        nc, in_maps, core_ids=list(range(N_CORES)), trace=_trace
    )
    idx = np.concatenate([r["out_idx"] for r in res.results], axis=0)
    resid = np.concatenate([r["out_res"] for r in res.results], axis=0)
    out = (idx.astype(np.int32), resid)
    if _trace:
        return out, res
    return out


# revision 18
# speedup vs baseline: 1.4836x; 1.0070x over previous
"""RQ-KMeans (residual vector quantization) Trainium2 kernel.

Problem: x [131072, 512] f32, codebooks [4, 1024, 512] f32.
Per book b: idx_b = argmin_k ||res - C_b[k]||^2 ; res -= C_b[idx_b].
Returns (idx [131072, 4] int32, res [131072, 512] f32).

Strategy: data-parallel over rows across 8 NeuronCores (16384 rows each),
codebooks replicated. Book-major on-device loop: for each book, stream row
tiles of the residual from DRAM, compute scores via PE matmul
(res . C^T - |C|^2/2), per-row argmax on DVE (max/max_index), gather the
selected code rows with indirect DMA, subtract, store the updated residual.
argmin(d2) == argmax(res.C - |C|^2/2) since the |res|^2 term is constant
per row.
"""

import sys

for _p in ("/root/.axon_site", "/root/.axon_site/_ro/trn_rl_repo", "/opt/trn_rl_repo"):
    if _p not in sys.path:
        sys.path.append(_p)

import numpy as np

import concourse.bass as bass
import concourse.mybir as mybir
import concourse.tile as tile
from concourse import bacc, bass_utils
from concourse.bass import ds
from concourse.bass_interp import get_hw_module
from concourse.masks import make_identity

N_CORES = 8
D = 512
K = 1024
NUM_BOOK = 4
P = 128
DCH = D // P  # 4 contraction chunks
KH = K // 512  # rhs free-dim halves (fp32 moving operand max is 512)

UNROLL = 32  # row tiles per For_i iteration
MM_DTYPE = mybir.dt.float32
# 3-pass float32r split matmul: res.C ~= rh.Ch + rh.Cl + rl.Ch with rh/rl
# (Ch/Cl) the f32r-rounded value and remainder. f32r streams 1 col/cycle vs
# fp32's 4, so 3 passes beat 1 fp32 pass by ~25%; dropped rl.Cl term is
# ~2^-22 relative — below fp32 accumulation-order noise.
SPLIT3 = bool(int(__import__("os").environ.get("RQ_SPLIT3", "0")))

_cache = {}


def _build(nc_rows: int):
    """Build + compile the per-core Bass program for nc_rows rows."""
    ntiles = nc_rows // P
    unroll = min(UNROLL, ntiles)
    assert ntiles % unroll == 0

    import time as _time

    _t0 = _time.time()
    nc = bacc.Bacc("TRN2", target_bir_lowering=False, debug=False)
    f32 = mybir.dt.float32

    x_d = nc.dram_tensor("x", [nc_rows, D], f32, kind="ExternalInput").ap()
    cbt_d = nc.dram_tensor("cbt", [NUM_BOOK, D, K], f32, kind="ExternalInput").ap()
    # [P, NUM_BOOK, K]: matches the SBUF tile layout (row-broadcast bias)
    cnorm_d = nc.dram_tensor("cnorm", [P, NUM_BOOK, K], f32, kind="ExternalInput").ap()
    # separate codebook tensors for the indirect gather (its source AP must
    # have offset 0, so one tensor per book)
    cbn_d = [
        nc.dram_tensor(f"cb{b}", [K, D], f32, kind="ExternalInput").ap()
        for b in range(NUM_BOOK)
    ]
    out_res_d = nc.dram_tensor("out_res", [nc_rows, D], f32, kind="ExternalOutput").ap()
    out_idx_d = nc.dram_tensor(
        "out_idx", [nc_rows, NUM_BOOK], mybir.dt.int32, kind="ExternalOutput"
    ).ap()
    res_scratch = [
        nc.dram_tensor(f"res_scratch{j}", [nc_rows, D], f32, kind="Internal").ap()
        for j in range(2)
    ]

    with tile.TileContext(nc, trace_sim=False) as tc:
        with (
            tc.tile_pool(name="const", bufs=1) as const_pool,
            tc.tile_pool(name="cbt", bufs=2) as cbt_pool,
            tc.tile_pool(name="work", bufs=min(6 if SPLIT3 else 10, 2 * unroll)) as work,
            tc.tile_pool(name="psum_t", bufs=2, space="PSUM") as psum_t,
            tc.tile_pool(name="psum_p", bufs=3, space="PSUM") as psum_p,
        ):
            ident = const_pool.tile([P, P], f32)
            make_identity(nc, ident[:])
            cnorm_sb = const_pool.tile([P, NUM_BOOK, K], f32)
            nc.sync.dma_start(cnorm_sb[:], cnorm_d)

            def body(b, src, dst, cbt_sb, row0):
                res_in = work.tile([P, D], f32, tag="res_in")
                nc.sync.dma_start(res_in[:], src[ds(row0, P), :])

                # resT in PSUM via 4 PE transposes
                resT_ps = psum_t.tile([P, D], f32, tag="resT")
                for c in range(DCH):
                    nc.tensor.transpose(
                        resT_ps[:, c * P : (c + 1) * P],
                        res_in[:, c * P : (c + 1) * P],
                        ident[:],
                    )
                p_ps = psum_p.tile([P, K], f32, tag="p")
                if SPLIT3:
                    f32r = mybir.dt.float32r
                    cbt_h, cbt_l = cbt_sb
                    rh_sb = work.tile([P, D], f32r, tag="rh_sb")
                    nc.vector.tensor_copy(rh_sb[:], resT_ps[:])
                    rl_sb = work.tile([P, D], f32r, tag="rl_sb")
                    nc.vector.tensor_sub(
                        rl_sb[:], resT_ps[:], rh_sb[:].bitcast(f32)
                    )
                    sets = [(rh_sb, cbt_h), (rh_sb, cbt_l), (rl_sb, cbt_h)]
                    for c in range(DCH):
                        for si, (lhs, rhs) in enumerate(sets):
                            for h in range(KH):
                                nc.tensor.matmul(
                                    out=p_ps[:, h * 512 : (h + 1) * 512],
                                    lhsT=lhs[:, c * P : (c + 1) * P],
                                    rhs=rhs[:, c, h * 512 : (h + 1) * 512],
                                    start=(c == 0 and si == 0),
                                    stop=(c == DCH - 1 and si == len(sets) - 1),
                                )
                else:
                    # scores: P_ps[n, k] = res_n . C_k  (accumulate over chunks)
                    resT_sb = work.tile([P, D], MM_DTYPE, tag="resT_sb")
                    nc.scalar.copy(resT_sb[:], resT_ps[:])
                    for c in range(DCH):
                        for h in range(KH):
                            nc.tensor.matmul(
                                out=p_ps[:, h * 512 : (h + 1) * 512],
                                lhsT=resT_sb[:, c * P : (c + 1) * P],
                                rhs=cbt_sb[:, c, h * 512 : (h + 1) * 512],
                                start=(c == 0),
                                stop=(c == DCH - 1),
                            )

                # m = P - |C|^2/2 ; argmax over k
                m_sb = work.tile([P, K], f32, tag="m")
                nc.vector.tensor_sub(m_sb[:], p_ps[:], cnorm_sb[:, b, :])
                mx8 = work.tile([P, 8], f32, tag="mx8")
                nc.vector.max(out=mx8[:], in_=m_sb[:])
                idx8 = work.tile([P, 8], mybir.dt.uint32, tag="idx8")
                nc.vector.max_index(idx8[:], mx8[:], m_sb[:])

                # gather selected code rows and subtract
                csel = work.tile([P, D], f32, tag="csel")
                nc.gpsimd.indirect_dma_start(
                    out=csel[:],
                    out_offset=None,
                    in_=cbn_d[b],
                    in_offset=bass.IndirectOffsetOnAxis(ap=idx8[:, :1], axis=0),
                )
                res_out = work.tile([P, D], f32, tag="res_out")
                nc.vector.tensor_sub(res_out[:], res_in[:], csel[:])

                nc.sync.dma_start(dst[ds(row0, P), :], res_out[:])
                nc.sync.dma_start(
                    out_idx_d[ds(row0, P), b : b + 1],
                    idx8[:, :1].bitcast(mybir.dt.int32),
                )

            for b in range(NUM_BOOK):
                src = x_d if b == 0 else res_scratch[(b - 1) % 2]
                dst = out_res_d if b == NUM_BOOK - 1 else res_scratch[b % 2]
                if SPLIT3:
                    f32r = mybir.dt.float32r
                    cbt_h = cbt_pool.tile([P, DCH, K], f32r, tag="cbt_h")
                    cbt_l = cbt_pool.tile([P, DCH, K], f32r, tag="cbt_l")
                    for c in range(DCH):
                        tmp = work.tile([P, K], f32, tag="cbt_tmp")
                        nc.sync.dma_start(
                            tmp[:], cbt_d[b, c * P : (c + 1) * P, :]
                        )
                        nc.vector.tensor_copy(cbt_h[:, c, :], tmp[:])
                        nc.vector.tensor_sub(
                            cbt_l[:, c, :], tmp[:], cbt_h[:, c, :].bitcast(f32)
                        )
                    cbt_sb = (cbt_h, cbt_l)
                else:
                    cbt_sb = cbt_pool.tile([P, DCH, K], MM_DTYPE, tag="cbt_sb")
                    nc.sync.dma_start(
                        cbt_sb[:], cbt_d[b].rearrange("(c p) k -> p c k", p=P)
                    )
                if ntiles == unroll:
                    for u in range(unroll):
                        body(b, src, dst, cbt_sb, u * P)
                else:
                    with tc.For_i(0, ntiles * P, unroll * P) as row0:
                        for u in range(unroll):
                            body(b, src, dst, cbt_sb, row0 + u * P)

    import time as _time

    _t = _time.time()
    nc.compile()
    print(f"[build] bacc compile: {_time.time()-_t:.1f}s", flush=True)
    nc.m = get_hw_module(nc.m)
    return nc


def _cnorm_like_reference(codebooks: np.ndarray) -> np.ndarray:
    """|C_k|^2 computed to match the reference's jnp.sum(C*C, -1) on XLA-CPU
    bit-for-bit (the reference can only run on CPU: its argmin reduce doesn't
    compile for the neuron device). Runs in a clean subprocess so it works
    even when the calling process's jax is locked to the axon platform."""
    import subprocess, tempfile, os

    try:
        with tempfile.TemporaryDirectory() as td:
            src = os.path.join(td, "cb.npy")
            dst = os.path.join(td, "cn.npy")
            np.save(src, codebooks)
            env = {k: v for k, v in os.environ.items() if k != "TRN_TERMINAL_POOL_IPS"}
            env["JAX_PLATFORMS"] = "cpu"
            code = (
                "import numpy as np, jax.numpy as jnp;"
                f"c = jnp.asarray(np.load({src!r}));"
                f"np.save({dst!r}, np.asarray(jnp.sum(c * c, axis=-1), dtype=np.float32))"
            )
            subprocess.run(
                [sys.executable, "-c", code], env=env, check=True,
                capture_output=True, timeout=300,
            )
            return np.load(dst)
    except Exception:
        pass
    try:
        import jax
        import jax.numpy as jnp

        cpu = jax.devices("cpu")[0]
        with jax.default_device(cpu):
            cb_j = jax.device_put(codebooks, cpu)
            return np.asarray(jnp.sum(cb_j * cb_j, axis=-1), dtype=np.float32)
    except Exception:
        return np.sum(codebooks.astype(np.float32) ** 2, axis=-1, dtype=np.float32)


def _prep_inputs(x: np.ndarray, codebooks: np.ndarray):
    n = x.shape[0]
    nc_rows = n // N_CORES
    x = np.ascontiguousarray(x, dtype=np.float32)
    codebooks = np.ascontiguousarray(codebooks, dtype=np.float32)
    cbt = np.ascontiguousarray(codebooks.transpose(0, 2, 1))  # [B, D, K]
    cn = _cnorm_like_reference(codebooks)
    cnorm = np.ascontiguousarray(
        np.broadcast_to((0.5 * cn)[None, :, :], (P, NUM_BOOK, K)), dtype=np.float32
    )  # [P, B, K] — matches the SBUF tile layout
    in_maps = []
    for c in range(N_CORES):
        m = {
            "x": x[c * nc_rows : (c + 1) * nc_rows],
            "cbt": cbt,
            "cnorm": cnorm,
        }
        for b in range(NUM_BOOK):
            m[f"cb{b}"] = codebooks[b]
        in_maps.append(m)
    return in_maps, nc_rows


def kernel(x: np.ndarray, codebooks: np.ndarray, _trace=False):
    in_maps, nc_rows = _prep_inputs(x, codebooks)
    if nc_rows not in _cache:
        _cache[nc_rows] = _build(nc_rows)
    nc = _cache[nc_rows]
    res = bass_utils.run_bass_kernel_spmd(
        nc, in_maps, core_ids=list(range(N_CORES)), trace=_trace
    )
    idx = np.concatenate([r["out_idx"] for r in res.results], axis=0)
    resid = np.concatenate([r["out_res"] for r in res.results], axis=0)
    out = (idx.astype(np.int32), resid)
    if _trace:
        return out, res
    return out
